# revision 14
# baseline (speedup 1.0000x reference)
"""Trainium2 Bass kernel for MHA with query-axis softmax (nn_MHA_2568390443327).

Reference computation (B=4, N=2048, DIM=1024, 16 heads x 64):
    qkv = x @ w_qkv ; q,k,v = split(qkv)
    scores = (q @ k^T) * scale            # [b,h,i(query),j(key)]
    attn = softmax(scores, axis=QUERY)    # normalized over i, per key j
    y = attn @ v ; out = y @ w_out + b_out

Sharding (8 cores): batch (4) x head-half (2). Each core gets its batch's
x (pre-transposed), the qkv weight columns and w_out rows for its 8 heads,
and produces a partial [DIM, N] output (transposed, f16). Host sums the two
head-half partials per batch in f32 and transposes back.

Per-core schedule (the perf-critical part):
  - Scores are computed transposed S_T[j, i] so the query-axis softmax is a
    free-axis exp+row-sum on the Scalar engine (fused accumulator), and the
    1/denominator folds into a tiny per-row rescale of v.
  - The two heads of a pair run CONCURRENTLY on the PE via tile_position
    row-packing (scores, K=64) and col-packing (attn@v, M=64) -- measured
    ~1.75x on this hardware when the instructions' waits are pre-satisfied.
  - The main loop is software-pipelined over 64 (pair, j) steps with
    attn@v lagged 2 steps behind scores, so the PE never waits on the
    Scalar engine's exp chain; filler matmuls (v-proj, next pair's q/k
    proj, previous pair's out-proj partial) keep the PE dense so the HAM
    clock gate stays at 2.4 GHz.
  - Out-proj partials for pairs 0..2 are accumulated into SBUF (f16)
    during later pairs' attention, so the serial tail is only pair 3's
    32 matmuls + bias + DMA out.
"""

import os
import numpy as np

# ---------------------------------------------------------------------------
# Problem constants (hardcoded; kernel.py must be self-contained).
B = 4
N = 2048          # sequence length
F = 1024          # model dim (contraction for qkv proj)
HEADS_TOT = 16
DH = 64           # head dim
HH = 8            # heads per core (head-half)
CH = HH * DH      # 512: per-core hidden
OUT = 1024        # output dim
SCALE = 0.125     # 1/sqrt(64)
N_CORES = 8

P = 128           # partitions
NC512 = 512       # matmul free-dim chunk
S_W = 1024        # scores PSUM tile width (2 banks)
PAIRS = 4         # head pairs per core
NT = N // P       # 16 j-tiles
KT = F // P       # 8 k-tiles for qkv projection
OT = OUT // P     # 8 output row tiles
LAG = 2           # attn@v runs LAG steps behind scores


def _build_nc():
    import concourse.bass as bass  # noqa: F401
    import concourse.mybir as mybir
    from concourse import bacc
    from concourse.tile import TileContext

    f32 = mybir.dt.float32
    f16 = mybir.dt.float16
    EXP = mybir.ActivationFunctionType.Exp
    ADD = mybir.AluOpType.add
    AXY = mybir.AxisListType.XY

    nc = bacc.Bacc(None, target_bir_lowering=False)

    xT = nc.declare_dram_parameter("xT", [F, N], f16, isOutput=False)
    wqkv = nc.declare_dram_parameter("wqkv", [F, 3 * CH], f16, isOutput=False)
    wout = nc.declare_dram_parameter("wout", [CH, OUT], f16, isOutput=False)
    bias = nc.declare_dram_parameter("bias", [P, OUT // P], f32,
                                     isOutput=False)
    outT = nc.declare_dram_parameter("outT", [OUT, N], f16, isOutput=True)

    with TileContext(nc) as tc:
        with (
            tc.tile_pool(name="p_x", bufs=1) as p_x,
            tc.tile_pool(name="p_w", bufs=1) as p_w,
            tc.tile_pool(name="p_qkT", bufs=2) as p_qkT,
            tc.tile_pool(name="p_v", bufs=1) as p_v,
            tc.tile_pool(name="p_wout", bufs=1) as p_wout,
            tc.tile_pool(name="p_small", bufs=1) as p_small,
            tc.tile_pool(name="p_ysb", bufs=1) as p_ysb,
            tc.tile_pool(name="p_oacc", bufs=1) as p_oacc,
            tc.tile_pool(name="p_at", bufs=12) as p_at,
            tc.tile_pool(name="p_vp", bufs=8) as p_vp,
            tc.tile_pool(name="p_den", bufs=24) as p_den,
            tc.tile_pool(name="p_osb", bufs=4) as p_osb,
            tc.tile_pool(name="psMM", bufs=1, space="PSUM") as psMM,
            tc.tile_pool(name="psY", bufs=1, space="PSUM") as psY,
        ):
            xt = [p_x.tile([P, N], f16, tag=f"x{k}", name=f"x{k}")
                  for k in range(KT)]
            wt = [p_w.tile([P, 3 * CH], f16, tag=f"w{k}", name=f"w{k}")
                  for k in range(KT)]
            # q/k SBUF tiles rotate: only 2 pairs resident (current + next)
            qk_tiles = {}

            def get_qk(pr):
                if pr not in qk_tiles:
                    qk_tiles[pr] = (
                        p_qkT.tile([P, N], f16, tag="qT", name=f"qT{pr}"),
                        p_qkT.tile([P, N], f16, tag="kT", name=f"kT{pr}"))
                return qk_tiles[pr]
            vnat = [p_v.tile([P, CH], f16, tag=f"v{j}", name=f"v{j}")
                    for j in range(NT)]
            wout_sb = [p_wout.tile([P, OUT], f16, tag=f"wo{c}",
                                   name=f"wo{c}") for c in range(PAIRS)]
            y_sb = [p_ysb.tile([P, N], f16, tag=f"y{p_}", name=f"y{p_}")
                    for p_ in range(PAIRS)]
            oacc = [p_oacc.tile([P, N], f16, tag=f"oa{o}", name=f"oa{o}")
                    for o in range(OT)]
            bias_sb = p_small.tile([P, OUT // P], f32, tag="bias",
                                   name="bias_sb")

            # ---- input DMA: x/w k-tiles interleaved so the qk projection
            # matmuls pipeline right behind the transfers
            for k in range(KT):
                nc.sync.dma_start(out=xt[k], in_=xT[k * P:(k + 1) * P, :])
                nc.sync.dma_start(out=wt[k], in_=wqkv[k * P:(k + 1) * P, :])
            for c in range(PAIRS):
                nc.sync.dma_start(out=wout_sb[c],
                                  in_=wout[c * P:(c + 1) * P, :])
            nc.sync.dma_start(out=bias_sb, in_=bias[:, :])

            # ---- q/k projection half-group (pair, sec, hf, c2): 8 MMs
            def emit_qk_half(pr, sec, hf, c2):
                dst = get_qk(pr)[sec]
                ps = psMM.tile([P, NC512], f32, tag="mm",
                               name=f"qk{pr}{sec}{hf}{c2}")
                i0 = hf * S_W + c2 * NC512
                for k in range(KT):
                    nc.tensor.matmul(
                        ps,
                        lhsT=wt[k][:, sec * CH + pr * P:
                                   sec * CH + (pr + 1) * P],
                        rhs=xt[k][:, i0:i0 + NC512],
                        start=(k == 0), stop=(k == KT - 1))
                nc.vector.tensor_copy(dst[:, i0:i0 + NC512], ps)

            # two-group-interleaved variant used in the head so the k-loop
            # pipelines behind the x/w DMAs
            def emit_qk_groups_interleaved(specs):
                big = psMM.tile([P, 2 * S_W], f32, tag="mm", name="qkh")
                tiles = {gi: big[:, gi * S_W:(gi + 1) * S_W]
                         for gi in range(len(specs))}
                for k in range(KT):
                    for gi, (pr, sec, hf) in enumerate(specs):
                        for c2 in range(2):
                            nc.tensor.matmul(
                                tiles[gi][:, c2 * NC512:(c2 + 1) * NC512],
                                lhsT=wt[k][:, sec * CH + pr * P:
                                           sec * CH + (pr + 1) * P],
                                rhs=xt[k][:, hf * S_W + c2 * NC512:
                                          hf * S_W + (c2 + 1) * NC512],
                                start=(k == 0), stop=(k == KT - 1))
                for gi, (pr, sec, hf) in enumerate(specs):
                    dst = get_qk(pr)[sec]
                    nc.vector.tensor_copy(dst[:, hf * S_W:(hf + 1) * S_W],
                                          tiles[gi])

            # ---- v projection for one j-tile: [128, 512] psum
            def emit_v(j):
                ps = psMM.tile([P, NC512], f32, tag="mm", name=f"v{j}")
                for k in range(KT):
                    nc.tensor.matmul(
                        ps,
                        lhsT=xt[k][:, j * P:(j + 1) * P],
                        rhs=wt[k][:, 2 * CH:2 * CH + NC512],
                        start=(k == 0), stop=(k == KT - 1))
                nc.vector.tensor_copy(vnat[j], ps)

            # ---- scores + exp, one hf half (i-chunk of 1024) of step s.
            # Per half: two per-head [128,1024] psum tiles; the head-A/B MMs
            # are row-packed (tile_position 0/64) and emitted back-to-back
            # so they run concurrently once both slots are free.
            def emit_scores_half(st, hf):
                pr, j = st["pr"], st["j"]
                qTp, kTp = get_qk(pr)
                js = slice(j * P, (j + 1) * P)
                t2 = psMM.tile([P, 2 * S_W], f32, tag="mm", name=f"s{hf}")
                for c2 in range(2):
                    i0 = hf * S_W + c2 * NC512
                    nc.tensor.matmul(
                        t2[:, c2 * NC512:(c2 + 1) * NC512],
                        lhsT=kTp[0:DH, js],
                        rhs=qTp[0:DH, i0:i0 + NC512],
                        start=True, stop=True, tile_position=(0, 0))
                    nc.tensor.matmul(
                        t2[:, S_W + c2 * NC512:S_W + (c2 + 1) * NC512],
                        lhsT=kTp[DH:P, js],
                        rhs=qTp[DH:P, i0:i0 + NC512],
                        start=True, stop=True, tile_position=(64, 0))
                for ho in (0, 64):
                    at = p_at.tile([P, S_W], f16, tag="at",
                                   name=f"at{hf}{ho}")
                    den = p_den.tile([P, 1], f32, tag="den",
                                     name=f"dn{hf}{ho}")
                    nc.scalar.activation(
                        at, t2[:, (ho // 64) * S_W:(ho // 64 + 1) * S_W],
                        EXP, scale=SCALE, accum_out=den)
                    st["ats"][ho].append(at)
                    st["dens"][ho].append(den)

            # ---- denominator finalize + v rescale for step s (DVE)
            def emit_vp(st):
                j, pr = st["j"], st["pr"]
                st["vp"] = {}
                for ho in (0, 64):
                    dtot = p_den.tile([P, 1], f32, tag="den", name="dtot")
                    nc.vector.tensor_add(dtot, st["dens"][ho][0],
                                         st["dens"][ho][1])
                    rec = p_den.tile([P, 1], f32, tag="den", name="rec")
                    nc.vector.reciprocal(rec, dtot)
                    vp = p_vp.tile([P, DH], f16, tag="vp", name=f"vp{ho}")
                    c0 = pr * 2 * DH + ho
                    nc.vector.tensor_scalar_mul(
                        vp, vnat[j][:, c0:c0 + DH], rec)
                    st["vp"][ho] = vp

            # ---- attn@v, one hf half of a completed step (col-packed)
            def emit_attnv_half(st, y_ps, hf):
                j = st["j"]
                for c2 in range(2):
                    i0 = hf * S_W + c2 * NC512
                    cs = slice(c2 * NC512, (c2 + 1) * NC512)
                    for ho in (0, 64):
                        nc.tensor.matmul(
                            y_ps[ho:ho + DH, i0:i0 + NC512],
                            lhsT=st["vp"][ho],
                            rhs=st["ats"][ho][hf][:, cs],
                            start=(j == 0), stop=(j == NT - 1),
                            tile_position=(0, ho))

            # ---- out-projection for pairs {0,1}: accumulate both c-tiles
            # in psum, one DVE copy into oacc (f16 SBUF).
            # chunk = list of (o, ich)
            def emit_outproj01(chunk):
                for (o, ich) in chunk:
                    ps = psMM.tile([P, NC512], f32, tag="mm",
                                   name=f"po{o}{ich}")
                    ics = slice(ich * NC512, (ich + 1) * NC512)
                    for c in (0, 1):
                        nc.tensor.matmul(
                            ps,
                            lhsT=wout_sb[c][:, o * P:(o + 1) * P],
                            rhs=y_sb[c][:, ics],
                            start=(c == 0), stop=(c == 1))
                    nc.vector.tensor_copy(oacc[o][:, ics], ps)

            # ---- final out-projection (pairs {2,3}) + bias + oacc + DMA
            def emit_outproj_final(chunk):
                for (o, ich) in chunk:
                    ps = psMM.tile([P, NC512], f32, tag="mm",
                                   name=f"pf{o}{ich}")
                    ics = slice(ich * NC512, (ich + 1) * NC512)
                    for c in (2, 3):
                        nc.tensor.matmul(
                            ps,
                            lhsT=wout_sb[c][:, o * P:(o + 1) * P],
                            rhs=y_sb[c][:, ics],
                            start=(c == 2), stop=(c == 3))
                    osb = p_osb.tile([P, NC512], f16, tag="osb", name="osb")
                    nc.vector.scalar_tensor_tensor(
                        osb, ps, bias_sb[:, o:o + 1], oacc[o][:, ics],
                        op0=ADD, op1=ADD)
                    nc.sync.dma_start(
                        out=outT[o * P:(o + 1) * P, ics], in_=osb)

            # =========== head: qk projection for pair 0 + first v tiles
            emit_qk_groups_interleaved([(0, 0, 0), (0, 1, 0)])
            emit_qk_groups_interleaved([(0, 0, 1), (0, 1, 1)])
            for j in range(LAG):
                emit_v(j)

            # filler schedule: emitted after that step's attnv.
            # pair 0: v-proj j=2..15 + qk(1) 8 half-groups
            # pair 1: qk(2);  pair 2: qk(3) + outproj{0,1} start
            # pair 3: outproj{0,1} rest
            halves = [(sec, hf, c2) for sec in (0, 1) for hf in (0, 1)
                      for c2 in (0, 1)]
            fillers = {s: [] for s in range(64)}
            for jj in range(LAG, NT):          # v2..v15 at steps 0..13
                fillers[jj - LAG].append(lambda j=jj: emit_v(j))
            for g, (sec, hf, c2) in enumerate(halves):
                fillers[2 * g + 1].append(
                    lambda a=sec, b=hf, c=c2: emit_qk_half(1, a, b, c))
            for pr in (1, 2):
                for g, (sec, hf, c2) in enumerate(halves):
                    fillers[pr * NT + 2 * g + 1].append(
                        lambda a=sec, b=hf, c=c2, p_=pr + 1:
                        emit_qk_half(p_, a, b, c))
            oplist = [(o, ich) for o in range(OT) for ich in range(4)]
            for ci in range(8):                # 8 chunks of 4 units
                chunk = oplist[ci * 4:(ci + 1) * 4]
                s0 = 40 + 2 * ci if ci < 4 else 50 + 2 * (ci - 4)
                fillers[s0].append(
                    lambda c_=chunk: emit_outproj01(c_))

            # =========== main software-pipelined loop
            states = {}
            y_ps = None
            for s in range(64 + LAG):
                fl = fillers[s] if s < 64 else []
                if s < 64:
                    pr, j = divmod(s, NT)
                    states[s] = {"pr": pr, "j": j,
                                 "ats": {0: [], 64: []},
                                 "dens": {0: [], 64: []}}
                t = s - LAG
                st = states.pop(t) if t >= 0 else None
                if st is not None and st["j"] == 0:
                    y_ps = psY.tile([P, N], f32, tag="Y",
                                    name=f"yps{st['pr']}")
                # weave: scores half 0 | attnv half 0 | filler | scores
                # half 1 | attnv half 1 | filler | vp chain
                if s < 64:
                    emit_scores_half(states[s], 0)
                if st is not None:
                    emit_attnv_half(st, y_ps, 0)
                if fl:
                    fl[0]()
                if s < 64:
                    emit_scores_half(states[s], 1)
                if st is not None:
                    emit_attnv_half(st, y_ps, 1)
                    if st["j"] == NT - 1:
                        nc.vector.tensor_copy(y_sb[st["pr"]], y_ps)
                for fn in fl[1:]:
                    fn()
                if s < 64:
                    emit_vp(states[s])

            # =========== tail: pair-3 out-projection
            for ci in range(4):
                emit_outproj_final(oplist[ci * 8:(ci + 1) * 8])
    return nc


def _shard_inputs(x, w_qkv, w_out, b_out):
    """Build per-core input maps: core c -> (batch c//2, head-half c%2)."""
    in_maps = []
    for c in range(N_CORES):
        b, hh = c // 2, c % 2
        cols = slice(hh * CH, (hh + 1) * CH)
        xTc = np.ascontiguousarray(np.asarray(x[b]).T, dtype=np.float16)
        wq = w_qkv[:, 0 * F:1 * F][:, cols]
        wk = w_qkv[:, 1 * F:2 * F][:, cols]
        wv = w_qkv[:, 2 * F:3 * F][:, cols]
        wqkv_c = np.ascontiguousarray(
            np.concatenate([wq, wk, wv], axis=1), dtype=np.float16)
        wout_c = np.ascontiguousarray(w_out[cols, :], dtype=np.float16)
        bias_c = np.ascontiguousarray(
            (np.asarray(b_out, dtype=np.float32) / 2.0)
            .reshape(OUT // P, P).T)
        in_maps.append(
            {"xT": xTc, "wqkv": wqkv_c, "wout": wout_c, "bias": bias_c})
    return in_maps


def _gather_outputs(results):
    out = np.empty((B, N, OUT), np.float32)
    for b in range(B):
        acc = (results[2 * b]["outT"].astype(np.float32)
               + results[2 * b + 1]["outT"].astype(np.float32))  # [OUT, N]
        out[b] = acc.T
    return out


# Test instrumentation (harness just calls kernel(); these stay default).
_TRACE = False
_LAST_RESULT = None


def kernel(x, w_qkv, w_out, b_out):
    global _LAST_RESULT
    # The bass->PJRT path needs the axon trn2 devices visible to jax.
    if os.environ.get("JAX_PLATFORMS") not in (None, "", "axon"):
        os.environ.pop("JAX_PLATFORMS", None)
    from concourse.bass_utils import run_bass_kernel_spmd

    nc = _build_nc()
    if not nc.is_finalized():
        nc.finalize()  # runs Bacc legalization (wait splitting, reg alloc)
    in_maps = _shard_inputs(np.asarray(x), np.asarray(w_qkv),
                            np.asarray(w_out), np.asarray(b_out))
    res = run_bass_kernel_spmd(nc, in_maps, list(range(N_CORES)),
                               trace=_TRACE)
    _LAST_RESULT = res
    return _gather_outputs(res.results)


# ---------------------------------------------------------------------------
# Numpy emulation of the per-core device program (for host-logic testing;
# not used by kernel()).
def _emulate_core(m):
    xT = m["xT"].astype(np.float32)
    wqkv, wout, bias = m["wqkv"], m["wout"], m["bias"]
    qT = (wqkv[:, 0:CH].T.astype(np.float32) @ xT)          # [CH, N]
    kTm = (wqkv[:, CH:2 * CH].T.astype(np.float32) @ xT)    # [CH, N]
    v = xT.T @ wqkv[:, 2 * CH:3 * CH].astype(np.float32)    # [N, CH]
    y = np.empty((CH, N), np.float32)
    for h in range(HH):
        qh = qT[h * DH:(h + 1) * DH, :]      # [DH, N(i)]
        kh = kTm[h * DH:(h + 1) * DH, :]     # [DH, N(j)]
        sT = kh.T @ qh                       # [j, i]
        e = np.exp(sT * SCALE)
        den = e.sum(axis=1, keepdims=True)   # over queries i, per key j
        vp = v[:, h * DH:(h + 1) * DH] / den
        y[h * DH:(h + 1) * DH, :] = vp.T @ e  # [DH, i]
    outT_acc = wout.T.astype(np.float32) @ y  # [OUT, N]
    outT_acc += bias.T.reshape(OUT, 1)
    return outT_acc.astype(np.float16)


def _kernel_emulated(x, w_qkv, w_out, b_out):
    in_maps = _shard_inputs(np.asarray(x), np.asarray(w_qkv),
                            np.asarray(w_out), np.asarray(b_out))
    results = [{"outT": _emulate_core(m)} for m in in_maps]
    return _gather_outputs(results)


# revision 16
# speedup vs baseline: 1.2321x; 1.2321x over previous
"""Trainium2 Bass kernel for MHA with query-axis softmax (nn_MHA_2568390443327).

Reference computation (B=4, N=2048, DIM=1024, 16 heads x 64):
    qkv = x @ w_qkv ; q,k,v = split(qkv)
    scores = (q @ k^T) * scale            # [b,h,i(query),j(key)]
    attn = softmax(scores, axis=QUERY)    # normalized over i, per key j
    y = attn @ v ; out = y @ w_out + b_out

Sharding (8 cores): batch (4) x head-half (2). Each core gets its batch's
x (pre-transposed), the qkv weight columns and w_out rows for its 8 heads,
and produces a partial [DIM, N] output (transposed, f16). Host sums the two
head-half partials per batch in f32 and transposes back.

Per-core schedule (the perf-critical part):
  - Scores are computed transposed S_T[j, i] so the query-axis softmax is a
    free-axis exp+row-sum on the Scalar engine (fused accumulator), and the
    1/denominator folds into a tiny per-row rescale of v.
  - The two heads of a pair run CONCURRENTLY on the PE via tile_position
    row-packing (scores, K=64) and col-packing (attn@v, M=64) -- measured
    ~1.75x on this hardware when the instructions' waits are pre-satisfied.
  - The main loop is software-pipelined over 64 (pair, j) steps with
    attn@v lagged 2 steps behind scores, so the PE never waits on the
    Scalar engine's exp chain; filler matmuls (v-proj, next pair's q/k
    proj, previous pair's out-proj partial) keep the PE dense so the HAM
    clock gate stays at 2.4 GHz.
  - Out-proj partials for pairs 0..2 are accumulated into SBUF (f16)
    during later pairs' attention, so the serial tail is only pair 3's
    32 matmuls + bias + DMA out.
"""

import os
import numpy as np

# ---------------------------------------------------------------------------
# Problem constants (hardcoded; kernel.py must be self-contained).
B = 4
N = 2048          # sequence length
F = 1024          # model dim (contraction for qkv proj)
HEADS_TOT = 16
DH = 64           # head dim
HH = 8            # heads per core (head-half)
CH = HH * DH      # 512: per-core hidden
OUT = 1024        # output dim
SCALE = 0.125     # 1/sqrt(64)
N_CORES = 8

P = 128           # partitions
NC512 = 512       # matmul free-dim chunk
S_W = 1024        # scores PSUM tile width (2 banks)
PAIRS = 4         # head pairs per core
NT = N // P       # 16 j-tiles
KT = F // P       # 8 k-tiles for qkv projection
OT = OUT // P     # 8 output row tiles
LAG = 2           # attn@v runs LAG steps behind scores


def _build_nc():
    import concourse.bass as bass  # noqa: F401
    import concourse.mybir as mybir
    from concourse import bacc
    from concourse.tile import TileContext

    f32 = mybir.dt.float32
    f16 = mybir.dt.float16
    EXP = mybir.ActivationFunctionType.Exp
    ADD = mybir.AluOpType.add
    AXY = mybir.AxisListType.XY

    nc = bacc.Bacc(None, target_bir_lowering=False)

    xT = nc.declare_dram_parameter("xT", [F, N], f16, isOutput=False)
    wqkv = nc.declare_dram_parameter("wqkv", [F, 3 * CH], f16, isOutput=False)
    wout = nc.declare_dram_parameter("wout", [CH, OUT], f16, isOutput=False)
    bias = nc.declare_dram_parameter("bias", [P, OUT // P], f32,
                                     isOutput=False)
    outT = nc.declare_dram_parameter("outT", [OUT, N], f16, isOutput=True)

    with TileContext(nc) as tc:
        with (
            tc.tile_pool(name="p_x", bufs=1) as p_x,
            tc.tile_pool(name="p_w", bufs=1) as p_w,
            tc.tile_pool(name="p_qkT", bufs=2) as p_qkT,
            tc.tile_pool(name="p_v", bufs=1) as p_v,
            tc.tile_pool(name="p_wout", bufs=1) as p_wout,
            tc.tile_pool(name="p_small", bufs=1) as p_small,
            tc.tile_pool(name="p_ysb", bufs=1) as p_ysb,
            tc.tile_pool(name="p_oacc", bufs=1) as p_oacc,
            tc.tile_pool(name="p_at", bufs=12) as p_at,
            tc.tile_pool(name="p_vp", bufs=8) as p_vp,
            tc.tile_pool(name="p_den", bufs=24) as p_den,
            tc.tile_pool(name="p_osb", bufs=4) as p_osb,
            tc.tile_pool(name="p_scr", bufs=6) as p_scr,
            tc.tile_pool(name="psMM", bufs=2, space="PSUM") as psMM,
            tc.tile_pool(name="psY", bufs=1, space="PSUM") as psY,
        ):
            # Mirror psMM's round-robin slot rotation so a pair's
            # first MM can also wait on the second slot's release -- the
            # two tile_position-packed MMs then issue together and run
            # concurrently on the PE array.
            from concourse.tile_rust import add_dep_helper
            slot_state = {"last": [None, None], "idx": 0, "pending": []}

            def slot_alloc(shape, name):
                t = psMM.tile(shape, f32, tag="mm", name=name)
                i = slot_state["idx"] % 2
                slot_state["idx"] += 1
                slot_state["pending"].append(i)
                # the other slot's last consumer gates our partner tile
                other = slot_state["last"][1 - i]
                return t, other

            def slot_consumed(inst):
                i = slot_state["pending"].pop(0)
                slot_state["last"][i] = inst

            xt = [p_x.tile([P, N], f16, tag=f"x{k}", name=f"x{k}")
                  for k in range(KT)]
            wt = [p_w.tile([P, 3 * CH], f16, tag=f"w{k}", name=f"w{k}")
                  for k in range(KT)]
            # q/k SBUF tiles rotate: only 2 pairs resident (current + next)
            qk_tiles = {}

            def get_qk(pr):
                if pr not in qk_tiles:
                    qk_tiles[pr] = (
                        p_qkT.tile([P, N], f16, tag="qT", name=f"qT{pr}"),
                        p_qkT.tile([P, N], f16, tag="kT", name=f"kT{pr}"))
                return qk_tiles[pr]
            vnat = [p_v.tile([P, CH], f16, tag=f"v{j}", name=f"v{j}")
                    for j in range(NT)]
            wout_sb = [p_wout.tile([P, OUT], f16, tag=f"wo{c}",
                                   name=f"wo{c}") for c in range(PAIRS)]
            y_sb = [p_ysb.tile([P, N], f16, tag=f"y{p_}", name=f"y{p_}")
                    for p_ in range(PAIRS)]
            oacc = [p_oacc.tile([P, N], f16, tag=f"oa{o}", name=f"oa{o}")
                    for o in range(OT)]
            bias_sb = p_small.tile([P, OUT // P], f32, tag="bias",
                                   name="bias_sb")

            # ---- input DMA: x/w k-tiles interleaved so the qk projection
            # matmuls pipeline right behind the transfers
            for k in range(KT):
                nc.sync.dma_start(out=xt[k], in_=xT[k * P:(k + 1) * P, :])
                nc.sync.dma_start(out=wt[k], in_=wqkv[k * P:(k + 1) * P, :])
            for c in range(PAIRS):
                nc.sync.dma_start(out=wout_sb[c],
                                  in_=wout[c * P:(c + 1) * P, :])
            nc.sync.dma_start(out=bias_sb, in_=bias[:, :])

            # ---- q/k projection half-group (pair, sec, hf, c2): 8 MMs
            def emit_qk_half(pr, sec, hf, c2):
                dst = get_qk(pr)[sec]
                ps, _ = slot_alloc([P, NC512], f"qk{pr}{sec}{hf}{c2}")
                i0 = hf * S_W + c2 * NC512
                for k in range(KT):
                    nc.tensor.matmul(
                        ps,
                        lhsT=wt[k][:, sec * CH + pr * P:
                                   sec * CH + (pr + 1) * P],
                        rhs=xt[k][:, i0:i0 + NC512],
                        start=(k == 0), stop=(k == KT - 1))
                cp = nc.vector.tensor_copy(dst[:, i0:i0 + NC512], ps)
                slot_consumed(cp)

            # two-group-interleaved variant used in the head so the k-loop
            # pipelines behind the x/w DMAs
            def emit_qk_groups_interleaved(specs):
                tiles = {}
                for gi in range(len(specs)):
                    tiles[gi], _ = slot_alloc([P, S_W], f"qkh{gi}")
                for k in range(KT):
                    for gi, (pr, sec, hf) in enumerate(specs):
                        for c2 in range(2):
                            nc.tensor.matmul(
                                tiles[gi][:, c2 * NC512:(c2 + 1) * NC512],
                                lhsT=wt[k][:, sec * CH + pr * P:
                                           sec * CH + (pr + 1) * P],
                                rhs=xt[k][:, hf * S_W + c2 * NC512:
                                          hf * S_W + (c2 + 1) * NC512],
                                start=(k == 0), stop=(k == KT - 1))
                for gi, (pr, sec, hf) in enumerate(specs):
                    dst = get_qk(pr)[sec]
                    cp = nc.vector.tensor_copy(
                        dst[:, hf * S_W:(hf + 1) * S_W], tiles[gi])
                    slot_consumed(cp)

            # ---- v projection for one j-tile: [128, 512] psum
            def emit_v(j):
                ps, _ = slot_alloc([P, NC512], f"v{j}")
                for k in range(KT):
                    nc.tensor.matmul(
                        ps,
                        lhsT=xt[k][:, j * P:(j + 1) * P],
                        rhs=wt[k][:, 2 * CH:2 * CH + NC512],
                        start=(k == 0), stop=(k == KT - 1))
                cp = nc.vector.tensor_copy(vnat[j], ps)
                slot_consumed(cp)

            # ---- scores + exp, one hf half (i-chunk of 1024) of step s.
            # Per half: two per-head [128,1024] psum tiles; the head-A/B MMs
            # are row-packed (tile_position 0/64) and emitted back-to-back
            # so they run concurrently once both slots are free.
            def emit_scores_half(st, hf):
                pr, j = st["pr"], st["j"]
                qTp, kTp = get_qk(pr)
                js = slice(j * P, (j + 1) * P)
                tA, gateA = slot_alloc([P, S_W], f"sA{hf}")
                tB, gateB = slot_alloc([P, S_W], f"sB{hf}")
                mms = []
                for c2 in range(2):
                    cs = slice(c2 * NC512, (c2 + 1) * NC512)
                    i0 = hf * S_W + c2 * NC512
                    mA = nc.tensor.matmul(
                        tA[:, cs], lhsT=kTp[0:DH, js],
                        rhs=qTp[0:DH, i0:i0 + NC512],
                        start=True, stop=True, tile_position=(0, 0))
                    mB = nc.tensor.matmul(
                        tB[:, cs], lhsT=kTp[DH:P, js],
                        rhs=qTp[DH:P, i0:i0 + NC512],
                        start=True, stop=True, tile_position=(64, 0))
                    mms.append((mA, mB))
                # co-issue: A-MMs also wait for B's slot to be free
                if gateA is not None:
                    add_dep_helper(gateA.ins, mms[0][0].ins, sync=True,
                                   reason="pair co-issue")
                tiles_ho = ((0, tA), (64, tB))
                for ho, t in tiles_ho:
                    at = p_at.tile([P, S_W], f16, tag="at",
                                   name=f"at{hf}{ho}")
                    den = p_den.tile([P, 1], f32, tag="den",
                                     name=f"dn{hf}{ho}")
                    ex = nc.scalar.activation(at, t, EXP, scale=SCALE,
                                              accum_out=den)
                    slot_consumed(ex)
                    st["ats"][ho].append(at)
                    st["dens"][ho].append(den)

            # ---- denominator finalize + v rescale for step s (DVE)
            def emit_vp(st):
                j, pr = st["j"], st["pr"]
                st["vp"] = {}
                for ho in (0, 64):
                    dtot = p_den.tile([P, 1], f32, tag="den", name="dtot")
                    nc.vector.tensor_add(dtot, st["dens"][ho][0],
                                         st["dens"][ho][1])
                    rec = p_den.tile([P, 1], f32, tag="den", name="rec")
                    nc.vector.reciprocal(rec, dtot)
                    vp = p_vp.tile([P, DH], f16, tag="vp", name=f"vp{ho}")
                    c0 = pr * 2 * DH + ho
                    nc.vector.tensor_scalar_mul(
                        vp, vnat[j][:, c0:c0 + DH], rec)
                    st["vp"][ho] = vp

            # ---- attn@v, one hf half of a completed step (col-packed)
            def emit_attnv_half(st, y_ps, hf):
                j = st["j"]
                for c2 in range(2):
                    i0 = hf * S_W + c2 * NC512
                    cs = slice(c2 * NC512, (c2 + 1) * NC512)
                    for ho in (0, 64):
                        nc.tensor.matmul(
                            y_ps[ho:ho + DH, i0:i0 + NC512],
                            lhsT=st["vp"][ho],
                            rhs=st["ats"][ho][hf][:, cs],
                            start=(j == 0), stop=(j == NT - 1),
                            tile_position=(0, ho))

            # ---- out-projection for pairs {0,1}: accumulate both c-tiles
            # in psum, one DVE copy into oacc (f16 SBUF).
            # chunk = list of (o, ich)
            def emit_outproj01(chunk):
                for (o, ich) in chunk:
                    ps, _ = slot_alloc([P, NC512], f"po{o}{ich}")
                    ics = slice(ich * NC512, (ich + 1) * NC512)
                    for c in (0, 1):
                        nc.tensor.matmul(
                            ps,
                            lhsT=wout_sb[c][:, o * P:(o + 1) * P],
                            rhs=y_sb[c][:, ics],
                            start=(c == 0), stop=(c == 1))
                    cp = nc.vector.tensor_copy(oacc[o][:, ics], ps)
                    slot_consumed(cp)

            # ---- final out-projection (pairs {2,3}) + bias + oacc + DMA
            def emit_outproj_final(chunk):
                for (o, ich) in chunk:
                    ps, _ = slot_alloc([P, NC512], f"pf{o}{ich}")
                    ics = slice(ich * NC512, (ich + 1) * NC512)
                    for c in (2, 3):
                        nc.tensor.matmul(
                            ps,
                            lhsT=wout_sb[c][:, o * P:(o + 1) * P],
                            rhs=y_sb[c][:, ics],
                            start=(c == 2), stop=(c == 3))
                    osb = p_osb.tile([P, NC512], f16, tag="osb", name="osb")
                    cp = nc.vector.scalar_tensor_tensor(
                        osb, ps, bias_sb[:, o:o + 1], oacc[o][:, ics],
                        op0=ADD, op1=ADD)
                    slot_consumed(cp)
                    nc.sync.dma_start(
                        out=outT[o * P:(o + 1) * P, ics], in_=osb)

            # =========== head: qk projection for pair 0 + first v tiles
            emit_qk_groups_interleaved([(0, 0, 0), (0, 1, 0)])
            emit_qk_groups_interleaved([(0, 0, 1), (0, 1, 1)])
            for j in range(LAG):
                emit_v(j)

            # filler schedule: emitted after that step's attnv.
            # pair 0: v-proj j=2..15 + qk(1) 8 half-groups
            # pair 1: qk(2);  pair 2: qk(3) + outproj{0,1} start
            # pair 3: outproj{0,1} rest
            halves = [(sec, hf, c2) for sec in (0, 1) for hf in (0, 1)
                      for c2 in (0, 1)]
            fillers = {s: [] for s in range(64)}
            for jj in range(LAG, NT):          # v2..v15 at steps 0..13
                fillers[jj - LAG].append(lambda j=jj: emit_v(j))
            for g, (sec, hf, c2) in enumerate(halves):
                fillers[2 * g + 1].append(
                    lambda a=sec, b=hf, c=c2: emit_qk_half(1, a, b, c))
            for pr in (1, 2):
                for g, (sec, hf, c2) in enumerate(halves):
                    fillers[pr * NT + 2 * g + 1].append(
                        lambda a=sec, b=hf, c=c2, p_=pr + 1:
                        emit_qk_half(p_, a, b, c))
            oplist = [(o, ich) for o in range(OT) for ich in range(4)]
            for ci in range(8):                # 8 chunks of 4 units
                chunk = oplist[ci * 4:(ci + 1) * 4]
                s0 = 40 + 2 * ci if ci < 4 else 50 + 2 * (ci - 4)
                fillers[s0].append(
                    lambda c_=chunk: emit_outproj01(c_))

            # =========== main software-pipelined loop
            states = {}
            y_ps = None
            for s in range(64 + LAG):
                fl = fillers[s] if s < 64 else []
                if s < 64:
                    pr, j = divmod(s, NT)
                    states[s] = {"pr": pr, "j": j,
                                 "ats": {0: [], 64: []},
                                 "dens": {0: [], 64: []}}
                t = s - LAG
                st = states.pop(t) if t >= 0 else None
                if st is not None and st["j"] == 0:
                    y_ps = psY.tile([P, N], f32, tag="Y",
                                    name=f"yps{st['pr']}")
                # weave: scores half 0 | attnv half 0 | filler | scores
                # half 1 | attnv half 1 | filler | vp chain
                if s < 64:
                    emit_scores_half(states[s], 0)
                if st is not None:
                    emit_attnv_half(st, y_ps, 0)
                if fl:
                    fl[0]()
                if s < 64:
                    emit_scores_half(states[s], 1)
                if st is not None:
                    emit_attnv_half(st, y_ps, 1)
                    if st["j"] == NT - 1:
                        nc.vector.tensor_copy(y_sb[st["pr"]], y_ps)
                for fn in fl[1:]:
                    fn()
                if s < 64:
                    emit_vp(states[s])

            # =========== tail: pair-3 out-projection
            for ci in range(4):
                emit_outproj_final(oplist[ci * 8:(ci + 1) * 8])
    return nc


def _shard_inputs(x, w_qkv, w_out, b_out):
    """Build per-core input maps: core c -> (batch c//2, head-half c%2)."""
    in_maps = []
    for c in range(N_CORES):
        b, hh = c // 2, c % 2
        cols = slice(hh * CH, (hh + 1) * CH)
        xTc = np.ascontiguousarray(np.asarray(x[b]).T, dtype=np.float16)
        wq = w_qkv[:, 0 * F:1 * F][:, cols]
        wk = w_qkv[:, 1 * F:2 * F][:, cols]
        wv = w_qkv[:, 2 * F:3 * F][:, cols]
        wqkv_c = np.ascontiguousarray(
            np.concatenate([wq, wk, wv], axis=1), dtype=np.float16)
        wout_c = np.ascontiguousarray(w_out[cols, :], dtype=np.float16)
        bias_c = np.ascontiguousarray(
            (np.asarray(b_out, dtype=np.float32) / 2.0)
            .reshape(OUT // P, P).T)
        in_maps.append(
            {"xT": xTc, "wqkv": wqkv_c, "wout": wout_c, "bias": bias_c})
    return in_maps


def _gather_outputs(results):
    out = np.empty((B, N, OUT), np.float32)
    for b in range(B):
        acc = (results[2 * b]["outT"].astype(np.float32)
               + results[2 * b + 1]["outT"].astype(np.float32))  # [OUT, N]
        out[b] = acc.T
    return out


# Test instrumentation (harness just calls kernel(); these stay default).
_TRACE = False
_LAST_RESULT = None


def kernel(x, w_qkv, w_out, b_out):
    global _LAST_RESULT
    # The bass->PJRT path needs the axon trn2 devices visible to jax.
    if os.environ.get("JAX_PLATFORMS") not in (None, "", "axon"):
        os.environ.pop("JAX_PLATFORMS", None)
    from concourse.bass_utils import run_bass_kernel_spmd

    nc = _build_nc()
    if not nc.is_finalized():
        nc.finalize()  # runs Bacc legalization (wait splitting, reg alloc)
    in_maps = _shard_inputs(np.asarray(x), np.asarray(w_qkv),
                            np.asarray(w_out), np.asarray(b_out))
    res = run_bass_kernel_spmd(nc, in_maps, list(range(N_CORES)),
                               trace=_TRACE)
    _LAST_RESULT = res
    return _gather_outputs(res.results)


# ---------------------------------------------------------------------------
# Numpy emulation of the per-core device program (for host-logic testing;
# not used by kernel()).
def _emulate_core(m):
    xT = m["xT"].astype(np.float32)
    wqkv, wout, bias = m["wqkv"], m["wout"], m["bias"]
    qT = (wqkv[:, 0:CH].T.astype(np.float32) @ xT)          # [CH, N]
    kTm = (wqkv[:, CH:2 * CH].T.astype(np.float32) @ xT)    # [CH, N]
    v = xT.T @ wqkv[:, 2 * CH:3 * CH].astype(np.float32)    # [N, CH]
    y = np.empty((CH, N), np.float32)
    for h in range(HH):
        qh = qT[h * DH:(h + 1) * DH, :]      # [DH, N(i)]
        kh = kTm[h * DH:(h + 1) * DH, :]     # [DH, N(j)]
        sT = kh.T @ qh                       # [j, i]
        e = np.exp(sT * SCALE)
        den = e.sum(axis=1, keepdims=True)   # over queries i, per key j
        vp = v[:, h * DH:(h + 1) * DH] / den
        y[h * DH:(h + 1) * DH, :] = vp.T @ e  # [DH, i]
    outT_acc = wout.T.astype(np.float32) @ y  # [OUT, N]
    outT_acc += bias.T.reshape(OUT, 1)
    return outT_acc.astype(np.float16)


def _kernel_emulated(x, w_qkv, w_out, b_out):
    in_maps = _shard_inputs(np.asarray(x), np.asarray(w_qkv),
                            np.asarray(w_out), np.asarray(b_out))
    results = [{"outT": _emulate_core(m)} for m in in_maps]
    return _gather_outputs(results)
